# revision 20
# baseline (speedup 1.0000x reference)
"""MultiHeadLatentAttention on 8 Trainium2 NeuronCores.

Sharding: 2 batches x 4 head-groups (4 heads each) = 8 cores.
Each core computes, for its batch b and heads [4*hg, 4*hg+4):
  q = x[b] @ Wq[:, cols]                  (computed transposed: qT [512, T])
  latent_new = LN(x[b] @ Wdown)           (computed transposed, replicated on
                                           the 4 cores of the same batch)
  kT = (latent @ Wk[:, cols]).T           v = latent @ Wv[:, cols]
  scores.T, softmax (no max-subtraction; |scores| <= ~3), PV accumulation
  o_partial = attn_out @ Wo[rows, :]      -> [T, D] partial sum
Host sums the 4 partials per batch and stacks the 2 batches.

Dtype strategy: fp32r (full-rate PE; its single-xbus LDWEIGHTS hides
fully: ~229 ns/matmul) for the projection phases; bf16 only for the
attention phase where SBUF capacity forces 2-byte K/V/q residency (bf16
pays ~+30 ns/matmul to FWL xbus contention).

Schedule notes: the down-projection runs before the q projection so the
tt=3 LayerNorm tail overlaps the q matmuls instead of stalling phase B.
x streams as [128,1024] tiles (8 matmuls per DMA descriptor) so the
sync-queue issue rate can keep the PE fed; weight/latent prefetches ride
the gpsimd DMA queue in parallel. V is computed for all 4 heads per
matmul (512-wide, full rate). The softmax denominator sums 4 exp'd
tiles on DVE per ones-matmul.
"""

import numpy as np

N_HEADS = 16
T = 2048
D = 2048
LAT = 512
PAST = 2048
S = PAST + T  # 4096, below the 8192 cache cap
HD = D // N_HEADS  # 128
HPC = 4  # heads per core
LN_EPS = 1e-5
SCALE = 1.0 / float(np.sqrt(HD))
NJB = S // 128  # 32 key blocks
NTT = T // 512  # 4 query tiles
NDC = D // 128  # 16
NLC = LAT // 128  # 4

_CACHE = {}


def _r(ap):
    import concourse.mybir as mybir

    return ap.bitcast(mybir.dt.float32r)


def _build():
    import concourse.bacc as bacc
    import concourse.mybir as mybir
    import concourse.tile as tile
    from concourse import bass_isa

    f32 = mybir.dt.float32
    f32r = mybir.dt.float32r
    bf16 = mybir.dt.bfloat16
    AF = mybir.ActivationFunctionType
    OP = mybir.AluOpType

    nc = bacc.Bacc("TRN2", target_bir_lowering=False, debug=False, num_devices=8)

    xT = nc.dram_tensor("xT", [D, T], f32, kind="ExternalInput")
    xq = nc.dram_tensor("xq", [D, 512], f32, kind="ExternalInput")
    lpT = nc.dram_tensor("lpT", [LAT, PAST], f32, kind="ExternalInput")
    wq = nc.dram_tensor("wq", [D, LAT], f32, kind="ExternalInput")
    wd = nc.dram_tensor("wd", [D, LAT], f32, kind="ExternalInput")
    wk = nc.dram_tensor("wk", [LAT, LAT], f32, kind="ExternalInput")
    wv = nc.dram_tensor("wv", [LAT, LAT], f32, kind="ExternalInput")
    wk16 = nc.dram_tensor("wk16", [LAT, LAT], bf16, kind="ExternalInput")
    wv16 = nc.dram_tensor("wv16", [LAT, LAT], bf16, kind="ExternalInput")
    wo = nc.dram_tensor("wo", [LAT, D], bf16, kind="ExternalInput")
    g = nc.dram_tensor("g", [LAT], f32, kind="ExternalInput")
    b = nc.dram_tensor("bb", [LAT], f32, kind="ExternalInput")
    mask = nc.dram_tensor("mask", [128, 896], bf16, kind="ExternalInput")
    o = nc.dram_tensor("o", [T, D], f32, kind="ExternalOutput")

    with tile.TileContext(nc) as tc:
        with (
            tc.tile_pool(name="consts", bufs=1) as consts,
            tc.tile_pool(name="persist", bufs=1) as persist,
            tc.tile_pool(name="ccd", bufs=1, space="DRAM") as ccd,
        ):
            # mask[jj, c] = 1 iff c >= jj + 384, so cols [512, 640) are all
            # ones for every partition — doubles as the ones matrix for the
            # softmax-denominator matmul.
            mask_sb = consts.tile([128, 896], bf16)
            nc.sync.dma_start(out=mask_sb, in_=mask[:, :])
            ones_sb = mask_sb[:, 512:640]
            g_sb = consts.tile([128, NLC], f32)
            nc.sync.dma_start(out=g_sb, in_=g[:].rearrange("(lc p) -> p lc", p=128))
            b_sb = consts.tile([128, NLC], f32)
            nc.sync.dma_start(out=b_sb, in_=b[:].rearrange("(lc p) -> p lc", p=128))
            eps_sb = consts.tile([128, 1], f32)
            nc.vector.memset(eps_sb, LN_EPS)

            qT_sb = persist.tile([128, HPC, T], bf16)  # q transposed, per head
            latn_sb = persist.tile([128, NLC, T], bf16)  # new latent (gathered)
            ao_sb = persist.tile([128, HPC, T], bf16)  # attn out transposed
            wk_sb = persist.tile([128, NLC, LAT], f32r)
            wv_sb = persist.tile([128, NLC, LAT], f32r)
            wk16_sb = persist.tile([128, NLC, LAT], bf16)
            wv16_sb = persist.tile([128, NLC, LAT], bf16)
            lp_sb = persist.tile([128, NLC, PAST], f32r)  # past latent
            # k/v weights ride the gpsimd DMA queue, in parallel with the
            # sync-queue weight/x streams of phase A
            nc.gpsimd.dma_start(
                wk_sb[:], wk[:, :].rearrange("(lc p) n -> p lc n", p=128).bitcast(f32r)
            )
            nc.gpsimd.dma_start(
                wv_sb[:], wv[:, :].rearrange("(lc p) n -> p lc n", p=128).bitcast(f32r)
            )
            nc.gpsimd.dma_start(
                wk16_sb[:], wk16[:, :].rearrange("(lc p) n -> p lc n", p=128)
            )
            nc.gpsimd.dma_start(
                wv16_sb[:], wv16[:, :].rearrange("(lc p) n -> p lc n", p=128)
            )
            cc_in = ccd.tile([LAT, 512], bf16)
            cc_out = ccd.tile([4 * LAT, 512], bf16)


            # ---- Phase A: latent down-projection + LayerNorm (first, so the
            # LN tail overlaps the q matmuls), then q projection.
            # The PSUM pool spans phases A and B: B's accumulators rotate
            # through the same tags, so B's first matmul only waits on one
            # q-copy instead of a full pool-close barrier.
            import contextlib as _ctx

            _es = _ctx.ExitStack()
            psA = _es.enter_context(tc.tile_pool(name="psA", bufs=1, space="PSUM"))
            with (
                tc.tile_pool(name="wA", bufs=1) as wA,
                tc.tile_pool(name="xa", bufs=5) as xa,
                tc.tile_pool(name="stats", bufs=1) as stats,
            ):
                wq_sb = wA.tile([128, NDC, LAT], f32r)

                latqb_sb = wA.tile([128, NLC, 512], bf16)

                def ln_ops_shard():
                    # LayerNorm over this core's latent quarter, staged as
                    # single-op thunks interleaved into the q-pass stream.
                    # Final scale+shift writes bf16 (the AllGather payload).
                    down = None
                    box = {}
                    ops = []

                    def alloc(nm, shape, tag):
                        box[nm] = stats.tile([128, shape], f32, tag=tag, name=nm)
                        return box[nm]

                    def dn(lc):
                        return latq_sb[:, lc, :]

                    ops.append(lambda: nc.vector.tensor_add(
                        alloc("ss2", 1024, "ss2")[:, 0:512], dn(0), dn(1)))
                    ops.append(lambda: nc.vector.tensor_add(
                        box["ss2"][:, 0:512], box["ss2"][:, 0:512], dn(2)))
                    ops.append(lambda: nc.vector.tensor_add(
                        box["ss2"][:, 0:512], box["ss2"][:, 0:512], dn(3)))
                    ops.append(lambda: nc.scalar.square(
                        alloc("sqa", 512, "sqa"), dn(0)))
                    ops.append(lambda: nc.scalar.square(
                        alloc("sqb", 512, "sqb"), dn(1)))
                    ops.append(lambda: nc.vector.tensor_add(
                        box["ss2"][:, 512:1024], box["sqa"], box["sqb"]))
                    ops.append(lambda: nc.scalar.square(
                        alloc("sqa2", 512, "sqa"), dn(2)))
                    ops.append(lambda: nc.vector.tensor_add(
                        box["ss2"][:, 512:1024], box["ss2"][:, 512:1024],
                        box["sqa2"]))
                    ops.append(lambda: nc.scalar.square(
                        alloc("sqb2", 512, "sqb"), dn(3)))
                    ops.append(lambda: nc.vector.tensor_add(
                        box["ss2"][:, 512:1024], box["ss2"][:, 512:1024],
                        box["sqb2"]))
                    ops.append(lambda: nc.gpsimd.partition_all_reduce(
                        alloc("pr2", 1024, "pr2"), box["ss2"], channels=128,
                        reduce_op=bass_isa.ReduceOp.add))
                    ops.append(lambda: nc.vector.tensor_scalar_mul(
                        alloc("mu", 512, "mu"), box["pr2"][:, 0:512], 1.0 / LAT))
                    ops.append(lambda: nc.vector.tensor_mul(
                        alloc("vtmp", 512, "vtmp"), box["mu"], box["mu"]))
                    ops.append(lambda: nc.vector.scalar_tensor_tensor(
                        out=alloc("sd", 512, "sd"), in0=box["pr2"][:, 512:1024],
                        scalar=1.0 / LAT, in1=box["vtmp"], op0=OP.mult,
                        op1=OP.subtract))
                    ops.append(lambda: nc.scalar.activation(
                        box["sd"], box["sd"], AF.Sqrt, bias=eps_sb))
                    ops.append(lambda: nc.vector.reciprocal_approx_fast(
                        alloc("rstd", 512, "rstd"), box["sd"]))
                    for lc in range(NLC):
                        ops.append(lambda lc=lc: nc.vector.tensor_sub(
                            alloc("t1", 512, "sqa"), dn(lc), box["mu"]))
                        ops.append(lambda: nc.vector.tensor_mul(
                            alloc("t2", 512, "sqb"), box["t1"], box["rstd"]))
                        ops.append(lambda lc=lc: nc.vector.tensor_scalar(
                            latqb_sb[:, lc, :], box["t2"], g_sb[:, lc : lc + 1],
                            b_sb[:, lc : lc + 1], OP.mult, OP.add))
                    return ops

                pend_ln = []

                def drain_ln(k):
                    for _ in range(min(k, len(pend_ln))):
                        pend_ln.pop(0)()

                # down-projection for this core's T/4 token quarter
                # (64 matmuls); the full new latent arrives via AllGather.
                latq_sb = wA.tile([128, NLC, 512], f32)
                d_ps = [
                    psA.tile([128, 512], f32, tag=f"t{i}", name=f"d_ps{i}")
                    for i in range(NLC)
                ]
                for dc in range(NDC):
                    xtq = xa.tile([128, 512], f32r, tag="xq", bufs=3, name="xtq")
                    nc.sync.dma_start(
                        out=xtq, in_=xq[dc * 128 : (dc + 1) * 128, :].bitcast(f32r)
                    )
                    wdt = xa.tile([128, 512], f32r, tag="wdt", bufs=3, name="wdt")
                    nc.sync.dma_start(
                        out=wdt, in_=wd[dc * 128 : (dc + 1) * 128, :].bitcast(f32r)
                    )
                    for lc in range(NLC):
                        nc.tensor.matmul(
                            d_ps[lc],
                            lhsT=_r(wdt[:, lc * 128 : (lc + 1) * 128]),
                            rhs=_r(xtq),
                            start=(dc == 0),
                            stop=(dc == NDC - 1),
                        )
                for lc in range(NLC):
                    if lc % 2 == 0:
                        nc.vector.tensor_copy(latq_sb[:, lc, :], d_ps[lc])
                    else:
                        nc.scalar.copy(out=latq_sb[:, lc, :], in_=d_ps[lc])
                for dc in range(4):
                    nc.sync.dma_start(
                        out=wq_sb[:, dc, :],
                        in_=wq[dc * 128 : (dc + 1) * 128, :].bitcast(f32r),
                    )
                pend_ln.extend(ln_ops_shard())

                # q projection (re-streams x; wq chunks 4..15 stream behind)
                for tp in range(2):
                    psl = slice(tp * 1024, (tp + 1) * 1024)
                    q_ps = [
                        psA.tile([128, 512], f32, tag=f"t{i}", name=f"q_ps{i}")
                        for i in range(8)
                    ]
                    for dc in range(NDC):
                        xt = xa.tile([128, 1024], f32r, tag="xt", name="xt")
                        nc.sync.dma_start(
                            out=xt,
                            in_=xT[dc * 128 : (dc + 1) * 128, psl].bitcast(f32r),
                        )
                        if tp == 0 and dc < 12:
                            nc.sync.dma_start(
                                out=wq_sb[:, dc + 4, :],
                                in_=wq[(dc + 4) * 128 : (dc + 5) * 128, :].bitcast(
                                    f32r
                                ),
                            )
                        drain_ln(4)
                        # spread the past-latent prefetch across the q-pass
                        # (lp_sb lives in persist: no pool barrier ahead of it)
                        lpc = tp * 16 + dc
                        if lpc % 2 == 0:
                            lpc //= 2
                            nc.sync.dma_start(
                                out=lp_sb[
                                    :,
                                    lpc % 4,
                                    (lpc // 4) * 512 : (lpc // 4 + 1) * 512,
                                ],
                                in_=lpT[
                                    (lpc % 4) * 128 : (lpc % 4 + 1) * 128,
                                    (lpc // 4) * 512 : (lpc // 4 + 1) * 512,
                                ].bitcast(f32r),
                            )
                        for half in range(2):
                            for qc in range(HPC):
                                nc.tensor.matmul(
                                    q_ps[half * 4 + qc],
                                    lhsT=_r(wq_sb[:, dc, qc * 128 : (qc + 1) * 128]),
                                    rhs=_r(xt[:, half * 512 : (half + 1) * 512]),
                                    start=(dc == 0),
                                    stop=(dc == NDC - 1),
                                )
                    for i in range(8):
                        half, qc = i // 4, i % 4
                        tt = tp * 2 + half
                        dst = qT_sb[:, qc, tt * 512 : (tt + 1) * 512]
                        if i % 2 == 0:
                            nc.scalar.copy(out=dst, in_=q_ps[i])
                        else:
                            nc.vector.tensor_copy(dst, q_ps[i])
                    if tp == 0:
                        drain_ln(999)
                        # ship this core's normalized latent quarter; gather
                        # the batch group's four quarters into latn_sb. The
                        # collective overlaps the rest of phase A and B-past.
                        nc.gpsimd.dma_start(
                            cc_in[:].rearrange("(lc p) t -> p lc t", p=128),
                            latqb_sb[:],
                        )
                        nc.gpsimd.collective_compute(
                            "AllGather",
                            mybir.AluOpType.bypass,
                            replica_groups=[[0, 1, 2, 3], [4, 5, 6, 7]],
                            ins=[cc_in.opt()],
                            outs=[cc_out.opt()],
                        )
                        for rk in range(4):
                            nc.gpsimd.dma_start(
                                latn_sb[:, :, rk * 512 : (rk + 1) * 512],
                                cc_out[rk * 512 : (rk + 1) * 512, :].rearrange(
                                    "(lc p) t -> p lc t", p=128
                                ),
                            )

            # ---- Phase B: k/v up-projection for all 4 heads
            with tc.tile_pool(name="kvbuf", bufs=1) as kvp:
                kT_sb = kvp.tile([128, HPC, S], bf16)
                v_sb = kvp.tile([128, NJB, LAT], bf16)
                if True:
                    krot = [0]
                    vrot = [0]

                    def latf(lc, g_):
                        if g_ < PAST // 512:
                            return lp_sb[:, lc, g_ * 512 : (g_ + 1) * 512]
                        gg = g_ - PAST // 512
                        return latn_sb[:, lc, gg * 512 : (gg + 1) * 512]

                    def wk_of(g_, lc, hsl):
                        if g_ < PAST // 512:
                            return _r(wk_sb[:, lc, hsl])
                        return wk16_sb[:, lc, hsl]

                    def wv_of(g_, lc):
                        if g_ < PAST // 512:
                            return _r(wv_sb[:, lc, :])
                        return wv16_sb[:, lc, :]

                    def lat_of(g_, lc, j4=None):
                        ap = latf(lc, g_)
                        if j4 is not None:
                            ap = ap[:, j4 * 128 : (j4 + 1) * 128]
                        if g_ < PAST // 512:
                            return _r(ap)
                        return ap

                    for g_ in range(S // 512):
                        ssl = slice(g_ * 512, (g_ + 1) * 512)
                        for h in range(HPC):
                            hsl = slice(h * 128, (h + 1) * 128)
                            k_ps = psA.tile(
                                [128, 512], f32, tag=f"t{krot[0]}", name="k_ps"
                            )
                            krot[0] = (krot[0] + 1) % 3
                            for lc in range(NLC):
                                nc.tensor.matmul(
                                    k_ps,
                                    lhsT=wk_of(g_, lc, hsl),
                                    rhs=lat_of(g_, lc),
                                    start=(lc == 0),
                                    stop=(lc == NLC - 1),
                                )
                            if h % 2 == 0:
                                nc.vector.tensor_copy(kT_sb[:, h, ssl], k_ps)
                            else:
                                nc.scalar.copy(out=kT_sb[:, h, ssl], in_=k_ps)
                        for j4 in range(4):
                            v_ps = psA.tile(
                                [128, 512], f32, tag=f"t{3 + vrot[0]}", name="v_ps"
                            )
                            vrot[0] = (vrot[0] + 1) % 3
                            for lc in range(NLC):
                                nc.tensor.matmul(
                                    v_ps,
                                    lhsT=lat_of(g_, lc, j4),
                                    rhs=wv_of(g_, lc),
                                    start=(lc == 0),
                                    stop=(lc == NLC - 1),
                                )
                            if j4 % 2 == 0:
                                nc.scalar.copy(out=v_sb[:, g_ * 4 + j4, :], in_=v_ps)
                            else:
                                nc.vector.tensor_copy(v_sb[:, g_ * 4 + j4, :], v_ps)

                _es.close()  # release the A/B PSUM banks for phase C

                # ---- Phase C: attention per head / query tile
                with (
                    tc.tile_pool(name="pp", bufs=6) as pp,
                    tc.tile_pool(name="pdp", bufs=3) as pdp,
                    tc.tile_pool(name="ctmp", bufs=2) as ctmp,
                    tc.tile_pool(name="psC", bufs=1, space="PSUM") as psC,
                ):
                  wo_sb = kvp.tile([128, HPC, D], bf16)
                  nc.sync.dma_start(
                      out=wo_sb[:],
                      in_=wo[:, :].rearrange("(hc p) n -> p hc n", p=128),
                  )
                  for h in range(HPC):
                    hsl = slice(h * 128, (h + 1) * 128)
                    for tt in range(NTT):
                        tsl = slice(tt * 512, (tt + 1) * 512)
                        nvis = PAST // 128 + 4 * (tt + 1)  # 20/24/28/32
                        # attention accumulator and softmax denominator share
                        # one psum tile (same lifetime)
                        ad = psC.tile([128, 1024], f32, tag="ad", bufs=2, name="ad")
                        attn_ps = ad[:, 0:512]
                        den_ps = ad[:, 512:1024]
                        pend = []
                        for jb in range(nvis):
                            s_ps = psC.tile(
                                [128, 512], f32, tag="sps", bufs=4, name="s_ps"
                            )
                            nc.tensor.matmul(
                                s_ps,
                                lhsT=kT_sb[:, h, jb * 128 : (jb + 1) * 128],
                                rhs=qT_sb[:, h, tsl],
                                start=True,
                                stop=True,
                            )
                            p = pp.tile([128, 512], bf16, tag="p", name="p")
                            nc.scalar.activation(p, s_ps, AF.Exp, scale=SCALE)
                            jbn = jb - PAST // 128
                            if jbn >= 0 and jbn // 4 == tt:
                                rr = (jbn % 4) * 128
                                nc.vector.tensor_mul(
                                    p, p, mask_sb[:, 384 - rr : 896 - rr]
                                )
                            nc.tensor.matmul(
                                attn_ps,
                                lhsT=v_sb[:, jb, hsl],
                                rhs=p,
                                start=(jb == 0),
                                stop=(jb == nvis - 1),
                            )
                            # denominator: tree-sum 4 exp'd tiles on DVE per
                            # ones-matmul
                            pend.append(p)
                            if len(pend) == 4:
                                e1 = pdp.tile([128, 512], bf16, tag="pd", name="e1")
                                nc.vector.tensor_add(e1, pend[0], pend[1])
                                e2 = pdp.tile([128, 512], bf16, tag="pd", name="e2")
                                nc.vector.tensor_add(e2, pend[2], pend[3])
                                e3 = pdp.tile([128, 512], bf16, tag="pd", name="e3")
                                nc.vector.tensor_add(e3, e1, e2)
                                nc.tensor.matmul(
                                    den_ps,
                                    lhsT=ones_sb,
                                    rhs=e3,
                                    start=(jb == 3),
                                    stop=(jb == nvis - 1),
                                )
                                pend.clear()
                        rec = ctmp.tile([128, 512], f32, tag="rec", name="rec")
                        nc.vector.reciprocal_approx_fast(rec, den_ps)
                        nc.vector.tensor_mul(ao_sb[:, h, tsl], attn_ps, rec)

                # ---- Phase D: output projection (bf16: ao and wo)
                with (
                    tc.tile_pool(name="ost", bufs=4) as ost,
                    tc.tile_pool(name="psD", bufs=4, space="PSUM") as psD,
                ):
                    for dt_ in range(D // 512):
                        for tc_ in range(T // 128):
                            o_ps = psD.tile([128, 512], f32, tag="ops", name="o_ps")
                            for hc in range(HPC):
                                nc.tensor.matmul(
                                    o_ps,
                                    lhsT=ao_sb[:, hc, tc_ * 128 : (tc_ + 1) * 128],
                                    rhs=wo_sb[:, hc, dt_ * 512 : (dt_ + 1) * 512],
                                    start=(hc == 0),
                                    stop=(hc == HPC - 1),
                                )
                            o_sb = ost.tile([128, 512], f32, tag="osb", name="o_sb")
                            if tc_ % 2 == 0:
                                nc.scalar.copy(out=o_sb, in_=o_ps)
                            else:
                                nc.vector.tensor_copy(o_sb, o_ps)
                            nc.sync.dma_start(
                                out=o[
                                    tc_ * 128 : (tc_ + 1) * 128,
                                    dt_ * 512 : (dt_ + 1) * 512,
                                ],
                                in_=o_sb,
                            )

    nc.compile()
    return nc


def _get_nc():
    if "nc" not in _CACHE:
        _CACHE["nc"] = _build()
    return _CACHE["nc"]


def _make_mask():
    # B[jj, c] = 1.0 iff c >= jj + 384; sliced at 384-r it gives the
    # causal staircase "visible iff i >= jj + r" for r in {0,128,256,384}.
    jj = np.arange(128)[:, None]
    cc = np.arange(896)[None, :]
    return (cc >= jj + 384)


def _in_maps(x, latent_prev, Wq, Wdown, Wk_up, Wv_up, ln_g, ln_b, Wo):
    import ml_dtypes

    bf = ml_dtypes.bfloat16
    f = lambda a: np.ascontiguousarray(np.asarray(a, dtype=np.float32))
    fb = lambda a: np.ascontiguousarray(np.asarray(a, dtype=np.float32)).astype(bf)
    mask = _make_mask().astype(bf)
    maps = []
    for bi in range(2):
        xTb = f(np.asarray(x)[bi].T)
        lpTb = f(np.asarray(latent_prev)[bi].T)
        wd_f = f(Wdown)
        for hg in range(4):
            sl = slice(hg * 512, (hg + 1) * 512)
            maps.append(
                {
                    "xT": xTb,
                    "xq": np.ascontiguousarray(xTb[:, sl]),
                    "lpT": lpTb,
                    "wq": f(np.asarray(Wq)[:, sl]),
                    "wd": wd_f,
                    "wk": f(np.asarray(Wk_up)[:, sl]),
                    "wv": f(np.asarray(Wv_up)[:, sl]),
                    "wk16": fb(np.asarray(Wk_up)[:, sl]),
                    "wv16": fb(np.asarray(Wv_up)[:, sl]),
                    "wo": fb(np.asarray(Wo)[sl, :]),
                    "g": f(ln_g),
                    "bb": f(ln_b),
                    "mask": mask,
                }
            )
    return maps


def run(trace=False, **inputs):
    from concourse.bass_utils import run_bass_kernel_spmd

    nc = _get_nc()
    maps = _in_maps(**inputs)
    res = run_bass_kernel_spmd(nc, maps, core_ids=list(range(8)), trace=trace)
    outs = [res.results[c]["o"] for c in range(8)]
    out = np.stack(
        [
            outs[0] + outs[1] + outs[2] + outs[3],
            outs[4] + outs[5] + outs[6] + outs[7],
        ],
        axis=0,
    ).astype(np.float32)
    return out, res


def kernel(**inputs):
    out, _ = run(trace=False, **inputs)
    return out


# revision 22
# speedup vs baseline: 1.0148x; 1.0148x over previous
"""MultiHeadLatentAttention on 8 Trainium2 NeuronCores.

Sharding: 2 batches x 4 head-groups (4 heads each) = 8 cores.
Each core computes, for its batch b and heads [4*hg, 4*hg+4):
  q = x[b] @ Wq[:, cols]                  (computed transposed: qT [512, T])
  latent_new = LN(x[b] @ Wdown)           (computed transposed, replicated on
                                           the 4 cores of the same batch)
  kT = (latent @ Wk[:, cols]).T           v = latent @ Wv[:, cols]
  scores.T, softmax (no max-subtraction; |scores| <= ~3), PV accumulation
  o_partial = attn_out @ Wo[rows, :]      -> [T, D] partial sum
Host sums the 4 partials per batch and stacks the 2 batches.

Dtype strategy: fp32r (full-rate PE; its single-xbus LDWEIGHTS hides
fully: ~229 ns/matmul) for the projection phases; bf16 only for the
attention phase where SBUF capacity forces 2-byte K/V/q residency (bf16
pays ~+30 ns/matmul to FWL xbus contention).

Schedule notes: the down-projection runs before the q projection so the
tt=3 LayerNorm tail overlaps the q matmuls instead of stalling phase B.
x streams as [128,1024] tiles (8 matmuls per DMA descriptor) so the
sync-queue issue rate can keep the PE fed; weight/latent prefetches ride
the gpsimd DMA queue in parallel. V is computed for all 4 heads per
matmul (512-wide, full rate). The softmax denominator sums 4 exp'd
tiles on DVE per ones-matmul.
"""

import numpy as np

N_HEADS = 16
T = 2048
D = 2048
LAT = 512
PAST = 2048
S = PAST + T  # 4096, below the 8192 cache cap
HD = D // N_HEADS  # 128
HPC = 4  # heads per core
LN_EPS = 1e-5
SCALE = 1.0 / float(np.sqrt(HD))
NJB = S // 128  # 32 key blocks
NTT = T // 512  # 4 query tiles
NDC = D // 128  # 16
NLC = LAT // 128  # 4

_CACHE = {}


def _r(ap):
    import concourse.mybir as mybir

    return ap.bitcast(mybir.dt.float32r)


def _build():
    import concourse.bacc as bacc
    import concourse.mybir as mybir
    import concourse.tile as tile
    from concourse import bass_isa

    f32 = mybir.dt.float32
    f32r = mybir.dt.float32r
    bf16 = mybir.dt.bfloat16
    AF = mybir.ActivationFunctionType
    OP = mybir.AluOpType

    nc = bacc.Bacc("TRN2", target_bir_lowering=False, debug=False, num_devices=8)

    xT = nc.dram_tensor("xT", [D, T], f32, kind="ExternalInput")
    xq = nc.dram_tensor("xq", [D, 512], f32, kind="ExternalInput")
    lpT = nc.dram_tensor("lpT", [LAT, PAST], f32, kind="ExternalInput")
    wq = nc.dram_tensor("wq", [D, LAT], f32, kind="ExternalInput")
    wd = nc.dram_tensor("wd", [D, LAT], f32, kind="ExternalInput")
    wk = nc.dram_tensor("wk", [LAT, LAT], f32, kind="ExternalInput")
    wv = nc.dram_tensor("wv", [LAT, LAT], f32, kind="ExternalInput")
    wk16 = nc.dram_tensor("wk16", [LAT, LAT], bf16, kind="ExternalInput")
    wv16 = nc.dram_tensor("wv16", [LAT, LAT], bf16, kind="ExternalInput")
    wo = nc.dram_tensor("wo", [LAT, D], bf16, kind="ExternalInput")
    g = nc.dram_tensor("g", [LAT], f32, kind="ExternalInput")
    b = nc.dram_tensor("bb", [LAT], f32, kind="ExternalInput")
    mask = nc.dram_tensor("mask", [128, 896], bf16, kind="ExternalInput")
    o = nc.dram_tensor("o", [T, D], f32, kind="ExternalOutput")

    with tile.TileContext(nc) as tc:
        with (
            tc.tile_pool(name="consts", bufs=1) as consts,
            tc.tile_pool(name="persist", bufs=1) as persist,
            tc.tile_pool(name="ccd", bufs=1, space="DRAM") as ccd,
        ):
            # mask[jj, c] = 1 iff c >= jj + 384, so cols [512, 640) are all
            # ones for every partition — doubles as the ones matrix for the
            # softmax-denominator matmul.
            mask_sb = consts.tile([128, 896], bf16)
            nc.sync.dma_start(out=mask_sb, in_=mask[:, :])
            ones_sb = mask_sb[:, 512:640]
            g_sb = consts.tile([128, NLC], f32)
            nc.sync.dma_start(out=g_sb, in_=g[:].rearrange("(lc p) -> p lc", p=128))
            b_sb = consts.tile([128, NLC], f32)
            nc.sync.dma_start(out=b_sb, in_=b[:].rearrange("(lc p) -> p lc", p=128))
            eps_sb = consts.tile([128, 1], f32)
            nc.vector.memset(eps_sb, LN_EPS)

            qT_sb = persist.tile([128, HPC, T], bf16)  # q transposed, per head
            latn_sb = persist.tile([128, NLC, T], bf16)  # new latent (gathered)
            ao_sb = persist.tile([128, HPC, T], bf16)  # attn out transposed
            wk_sb = persist.tile([128, NLC, LAT], f32r)
            wv_sb = persist.tile([128, NLC, LAT], f32r)
            wk16_sb = persist.tile([128, NLC, LAT], bf16)
            wv16_sb = persist.tile([128, NLC, LAT], bf16)
            # k/v weights ride the gpsimd DMA queue, in parallel with the
            # sync-queue weight/x streams of phase A
            nc.gpsimd.dma_start(
                wk_sb[:], wk[:, :].rearrange("(lc p) n -> p lc n", p=128).bitcast(f32r)
            )
            nc.gpsimd.dma_start(
                wv_sb[:], wv[:, :].rearrange("(lc p) n -> p lc n", p=128).bitcast(f32r)
            )
            nc.gpsimd.dma_start(
                wk16_sb[:], wk16[:, :].rearrange("(lc p) n -> p lc n", p=128)
            )
            nc.gpsimd.dma_start(
                wv16_sb[:], wv16[:, :].rearrange("(lc p) n -> p lc n", p=128)
            )
            cc_in = ccd.tile([LAT, 512], bf16)
            cc_out = ccd.tile([4 * LAT, 512], bf16)


            # ---- Phase A: latent down-projection + LayerNorm (first, so the
            # LN tail overlaps the q matmuls), then q projection.
            with (
                tc.tile_pool(name="wA", bufs=1) as wA,
                tc.tile_pool(name="xa", bufs=5) as xa,
                tc.tile_pool(name="stats", bufs=1) as stats,
                tc.tile_pool(name="psA", bufs=1, space="PSUM") as psA,
            ):
                wq_sb = wA.tile([128, NDC, LAT], f32r)
                wd_sb = wA.tile([128, NDC, LAT], f32r)
                for dc in range(4):
                    nc.sync.dma_start(
                        out=wd_sb[:, dc, :],
                        in_=wd[dc * 128 : (dc + 1) * 128, :].bitcast(f32r),
                    )

                latqb_sb = wA.tile([128, NLC, 512], bf16)

                def ln_ops_shard():
                    # LayerNorm over this core's latent quarter, staged as
                    # single-op thunks interleaved into the q-pass stream.
                    # Final scale+shift writes bf16 (the AllGather payload).
                    down = None
                    box = {}
                    ops = []

                    def alloc(nm, shape, tag):
                        box[nm] = stats.tile([128, shape], f32, tag=tag, name=nm)
                        return box[nm]

                    def dn(lc):
                        return latq_sb[:, lc, :]

                    ops.append(lambda: nc.vector.tensor_add(
                        alloc("ss2", 1024, "ss2")[:, 0:512], dn(0), dn(1)))
                    ops.append(lambda: nc.vector.tensor_add(
                        box["ss2"][:, 0:512], box["ss2"][:, 0:512], dn(2)))
                    ops.append(lambda: nc.vector.tensor_add(
                        box["ss2"][:, 0:512], box["ss2"][:, 0:512], dn(3)))
                    ops.append(lambda: nc.scalar.square(
                        alloc("sqa", 512, "sqa"), dn(0)))
                    ops.append(lambda: nc.scalar.square(
                        alloc("sqb", 512, "sqb"), dn(1)))
                    ops.append(lambda: nc.vector.tensor_add(
                        box["ss2"][:, 512:1024], box["sqa"], box["sqb"]))
                    ops.append(lambda: nc.scalar.square(
                        alloc("sqa2", 512, "sqa"), dn(2)))
                    ops.append(lambda: nc.vector.tensor_add(
                        box["ss2"][:, 512:1024], box["ss2"][:, 512:1024],
                        box["sqa2"]))
                    ops.append(lambda: nc.scalar.square(
                        alloc("sqb2", 512, "sqb"), dn(3)))
                    ops.append(lambda: nc.vector.tensor_add(
                        box["ss2"][:, 512:1024], box["ss2"][:, 512:1024],
                        box["sqb2"]))
                    ops.append(lambda: nc.gpsimd.partition_all_reduce(
                        alloc("pr2", 1024, "pr2"), box["ss2"], channels=128,
                        reduce_op=bass_isa.ReduceOp.add))
                    ops.append(lambda: nc.vector.tensor_scalar_mul(
                        alloc("mu", 512, "mu"), box["pr2"][:, 0:512], 1.0 / LAT))
                    ops.append(lambda: nc.vector.tensor_mul(
                        alloc("vtmp", 512, "vtmp"), box["mu"], box["mu"]))
                    ops.append(lambda: nc.vector.scalar_tensor_tensor(
                        out=alloc("sd", 512, "sd"), in0=box["pr2"][:, 512:1024],
                        scalar=1.0 / LAT, in1=box["vtmp"], op0=OP.mult,
                        op1=OP.subtract))
                    ops.append(lambda: nc.scalar.activation(
                        box["sd"], box["sd"], AF.Sqrt, bias=eps_sb))
                    ops.append(lambda: nc.vector.reciprocal_approx_fast(
                        alloc("rstd", 512, "rstd"), box["sd"]))
                    for lc in range(NLC):
                        ops.append(lambda lc=lc: nc.vector.tensor_sub(
                            alloc("t1", 512, "sqa"), dn(lc), box["mu"]))
                        ops.append(lambda: nc.vector.tensor_mul(
                            alloc("t2", 512, "sqb"), box["t1"], box["rstd"]))
                        ops.append(lambda lc=lc: nc.vector.tensor_scalar(
                            latqb_sb[:, lc, :], box["t2"], g_sb[:, lc : lc + 1],
                            b_sb[:, lc : lc + 1], OP.mult, OP.add))
                    return ops

                pend_ln = []

                def drain_ln(k):
                    for _ in range(min(k, len(pend_ln))):
                        pend_ln.pop(0)()

                # down-projection for this core's T/4 token quarter
                # (64 matmuls); the full new latent arrives via AllGather.
                latq_sb = wA.tile([128, NLC, 512], f32)
                d_ps = [
                    psA.tile([128, 512], f32, tag=f"t{i}", name=f"d_ps{i}")
                    for i in range(NLC)
                ]
                for dc in range(NDC):
                    xtq = xa.tile([128, 512], f32r, tag="xq", bufs=3, name="xtq")
                    nc.sync.dma_start(
                        out=xtq, in_=xq[dc * 128 : (dc + 1) * 128, :].bitcast(f32r)
                    )
                    if dc < 12:
                        nc.sync.dma_start(
                            out=wd_sb[:, dc + 4, :],
                            in_=wd[(dc + 4) * 128 : (dc + 5) * 128, :].bitcast(
                                f32r
                            ),
                        )
                    for lc in range(NLC):
                        nc.tensor.matmul(
                            d_ps[lc],
                            lhsT=_r(wd_sb[:, dc, lc * 128 : (lc + 1) * 128]),
                            rhs=_r(xtq),
                            start=(dc == 0),
                            stop=(dc == NDC - 1),
                        )
                for lc in range(NLC):
                    if lc % 2 == 0:
                        nc.vector.tensor_copy(latq_sb[:, lc, :], d_ps[lc])
                    else:
                        nc.scalar.copy(out=latq_sb[:, lc, :], in_=d_ps[lc])
                for dc in range(4):
                    nc.sync.dma_start(
                        out=wq_sb[:, dc, :],
                        in_=wq[dc * 128 : (dc + 1) * 128, :].bitcast(f32r),
                    )
                pend_ln.extend(ln_ops_shard())

                # q projection (re-streams x; wq chunks 4..15 stream behind)
                for tp in range(2):
                    psl = slice(tp * 1024, (tp + 1) * 1024)
                    q_ps = [
                        psA.tile([128, 512], f32, tag=f"t{i}", name=f"q_ps{i}")
                        for i in range(8)
                    ]
                    for dc in range(NDC):
                        xt = xa.tile([128, 1024], f32r, tag="xt", name="xt")
                        nc.sync.dma_start(
                            out=xt,
                            in_=xT[dc * 128 : (dc + 1) * 128, psl].bitcast(f32r),
                        )
                        if tp == 0 and dc < 12:
                            nc.sync.dma_start(
                                out=wq_sb[:, dc + 4, :],
                                in_=wq[(dc + 4) * 128 : (dc + 5) * 128, :].bitcast(
                                    f32r
                                ),
                            )
                        drain_ln(4)
                        for half in range(2):
                            for qc in range(HPC):
                                nc.tensor.matmul(
                                    q_ps[half * 4 + qc],
                                    lhsT=_r(wq_sb[:, dc, qc * 128 : (qc + 1) * 128]),
                                    rhs=_r(xt[:, half * 512 : (half + 1) * 512]),
                                    start=(dc == 0),
                                    stop=(dc == NDC - 1),
                                )
                    for i in range(8):
                        half, qc = i // 4, i % 4
                        tt = tp * 2 + half
                        dst = qT_sb[:, qc, tt * 512 : (tt + 1) * 512]
                        if i % 2 == 0:
                            nc.scalar.copy(out=dst, in_=q_ps[i])
                        else:
                            nc.vector.tensor_copy(dst, q_ps[i])
                    if tp == 0:
                        drain_ln(999)
                        # ship this core's normalized latent quarter; gather
                        # the batch group's four quarters into latn_sb. The
                        # collective overlaps the rest of phase A and B-past.
                        nc.gpsimd.dma_start(
                            cc_in[:].rearrange("(lc p) t -> p lc t", p=128),
                            latqb_sb[:],
                        )
                        nc.gpsimd.collective_compute(
                            "AllGather",
                            mybir.AluOpType.bypass,
                            replica_groups=[[0, 1, 2, 3], [4, 5, 6, 7]],
                            ins=[cc_in.opt()],
                            outs=[cc_out.opt()],
                        )
                        for rk in range(4):
                            nc.gpsimd.dma_start(
                                latn_sb[:, :, rk * 512 : (rk + 1) * 512],
                                cc_out[rk * 512 : (rk + 1) * 512, :].rearrange(
                                    "(lc p) t -> p lc t", p=128
                                ),
                            )

            # ---- Phase B: k/v up-projection for all 4 heads
            with tc.tile_pool(name="kvbuf", bufs=1) as kvp:
                kT_sb = kvp.tile([128, HPC, S], bf16)
                v_sb = kvp.tile([128, NJB, LAT], bf16)
                with (
                    tc.tile_pool(name="bbuf", bufs=1) as bst,
                    tc.tile_pool(name="psB", bufs=1, space="PSUM") as psB,
                ):
                    lp_sb = bst.tile([128, NLC, PAST], f32r)

                    def latf(lc, g_):
                        if g_ < PAST // 512:
                            return lp_sb[:, lc, g_ * 512 : (g_ + 1) * 512]
                        gg = g_ - PAST // 512
                        return latn_sb[:, lc, gg * 512 : (gg + 1) * 512]

                    def wk_of(g_, lc, hsl):
                        if g_ < PAST // 512:
                            return _r(wk_sb[:, lc, hsl])
                        return wk16_sb[:, lc, hsl]

                    def wv_of(g_, lc):
                        if g_ < PAST // 512:
                            return _r(wv_sb[:, lc, :])
                        return wv16_sb[:, lc, :]

                    def lat_of(g_, lc, j4=None):
                        ap = latf(lc, g_)
                        if j4 is not None:
                            ap = ap[:, j4 * 128 : (j4 + 1) * 128]
                        if g_ < PAST // 512:
                            return _r(ap)
                        return ap

                    for g_ in range(S // 512):
                        ssl = slice(g_ * 512, (g_ + 1) * 512)
                        if g_ < PAST // 512:
                            # past-latent chunks stream on the sync queue
                            # (gpsimd is owned by the collective right now)
                            for lc in range(NLC):
                                nc.sync.dma_start(
                                    out=lp_sb[:, lc, ssl],
                                    in_=lpT[lc * 128 : (lc + 1) * 128, ssl].bitcast(
                                        f32r
                                    ),
                                )
                        for h in range(HPC):
                            hsl = slice(h * 128, (h + 1) * 128)
                            k_ps = psB.tile(
                                [128, 512], f32, tag="kps", bufs=3, name="k_ps"
                            )
                            for lc in range(NLC):
                                nc.tensor.matmul(
                                    k_ps,
                                    lhsT=wk_of(g_, lc, hsl),
                                    rhs=lat_of(g_, lc),
                                    start=(lc == 0),
                                    stop=(lc == NLC - 1),
                                )
                            if h % 2 == 0:
                                nc.vector.tensor_copy(kT_sb[:, h, ssl], k_ps)
                            else:
                                nc.scalar.copy(out=kT_sb[:, h, ssl], in_=k_ps)
                        for j4 in range(4):
                            v_ps = psB.tile(
                                [128, 512], f32, tag="vps", bufs=3, name="v_ps"
                            )
                            for lc in range(NLC):
                                nc.tensor.matmul(
                                    v_ps,
                                    lhsT=lat_of(g_, lc, j4),
                                    rhs=wv_of(g_, lc),
                                    start=(lc == 0),
                                    stop=(lc == NLC - 1),
                                )
                            if j4 % 2 == 0:
                                nc.scalar.copy(out=v_sb[:, g_ * 4 + j4, :], in_=v_ps)
                            else:
                                nc.vector.tensor_copy(v_sb[:, g_ * 4 + j4, :], v_ps)

                # ---- Phase C: attention per head / query tile
                with (
                    tc.tile_pool(name="pp", bufs=6) as pp,
                    tc.tile_pool(name="pdp", bufs=3) as pdp,
                    tc.tile_pool(name="ctmp", bufs=2) as ctmp,
                    tc.tile_pool(name="psC", bufs=1, space="PSUM") as psC,
                ):
                  wo_sb = kvp.tile([128, HPC, D], bf16)
                  nc.sync.dma_start(
                      out=wo_sb[:],
                      in_=wo[:, :].rearrange("(hc p) n -> p hc n", p=128),
                  )
                  for h in range(HPC):
                    hsl = slice(h * 128, (h + 1) * 128)
                    for tt in range(NTT):
                        tsl = slice(tt * 512, (tt + 1) * 512)
                        nvis = PAST // 128 + 4 * (tt + 1)  # 20/24/28/32
                        # attention accumulator and softmax denominator share
                        # one psum tile (same lifetime)
                        ad = psC.tile([128, 1024], f32, tag="ad", bufs=2, name="ad")
                        attn_ps = ad[:, 0:512]
                        den_ps = ad[:, 512:1024]
                        pend = []
                        for jb in range(nvis):
                            s_ps = psC.tile(
                                [128, 512], f32, tag="sps", bufs=4, name="s_ps"
                            )
                            nc.tensor.matmul(
                                s_ps,
                                lhsT=kT_sb[:, h, jb * 128 : (jb + 1) * 128],
                                rhs=qT_sb[:, h, tsl],
                                start=True,
                                stop=True,
                            )
                            p = pp.tile([128, 512], bf16, tag="p", name="p")
                            nc.scalar.activation(p, s_ps, AF.Exp, scale=SCALE)
                            jbn = jb - PAST // 128
                            if jbn >= 0 and jbn // 4 == tt:
                                rr = (jbn % 4) * 128
                                nc.vector.tensor_mul(
                                    p, p, mask_sb[:, 384 - rr : 896 - rr]
                                )
                            nc.tensor.matmul(
                                attn_ps,
                                lhsT=v_sb[:, jb, hsl],
                                rhs=p,
                                start=(jb == 0),
                                stop=(jb == nvis - 1),
                            )
                            # denominator: tree-sum 4 exp'd tiles on DVE per
                            # ones-matmul
                            pend.append(p)
                            if len(pend) == 4:
                                e1 = pdp.tile([128, 512], bf16, tag="pd", name="e1")
                                nc.vector.tensor_add(e1, pend[0], pend[1])
                                e2 = pdp.tile([128, 512], bf16, tag="pd", name="e2")
                                nc.vector.tensor_add(e2, pend[2], pend[3])
                                e3 = pdp.tile([128, 512], bf16, tag="pd", name="e3")
                                nc.vector.tensor_add(e3, e1, e2)
                                nc.tensor.matmul(
                                    den_ps,
                                    lhsT=ones_sb,
                                    rhs=e3,
                                    start=(jb == 3),
                                    stop=(jb == nvis - 1),
                                )
                                pend.clear()
                        rec = ctmp.tile([128, 512], f32, tag="rec", name="rec")
                        nc.vector.reciprocal_approx_fast(rec, den_ps)
                        nc.vector.tensor_mul(ao_sb[:, h, tsl], attn_ps, rec)

                # ---- Phase D: output projection (bf16: ao and wo)
                with (
                    tc.tile_pool(name="ost", bufs=4) as ost,
                    tc.tile_pool(name="psD", bufs=4, space="PSUM") as psD,
                ):
                    for dt_ in range(D // 512):
                        for tc_ in range(T // 128):
                            o_ps = psD.tile([128, 512], f32, tag="ops", name="o_ps")
                            for hc in range(HPC):
                                nc.tensor.matmul(
                                    o_ps,
                                    lhsT=ao_sb[:, hc, tc_ * 128 : (tc_ + 1) * 128],
                                    rhs=wo_sb[:, hc, dt_ * 512 : (dt_ + 1) * 512],
                                    start=(hc == 0),
                                    stop=(hc == HPC - 1),
                                )
                            o_sb = ost.tile([128, 512], f32, tag="osb", name="o_sb")
                            if tc_ % 2 == 0:
                                nc.scalar.copy(out=o_sb, in_=o_ps)
                            else:
                                nc.vector.tensor_copy(o_sb, o_ps)
                            nc.sync.dma_start(
                                out=o[
                                    tc_ * 128 : (tc_ + 1) * 128,
                                    dt_ * 512 : (dt_ + 1) * 512,
                                ],
                                in_=o_sb,
                            )

    nc.compile()
    return nc


def _get_nc():
    if "nc" not in _CACHE:
        _CACHE["nc"] = _build()
    return _CACHE["nc"]


def _make_mask():
    # B[jj, c] = 1.0 iff c >= jj + 384; sliced at 384-r it gives the
    # causal staircase "visible iff i >= jj + r" for r in {0,128,256,384}.
    jj = np.arange(128)[:, None]
    cc = np.arange(896)[None, :]
    return (cc >= jj + 384)


def _in_maps(x, latent_prev, Wq, Wdown, Wk_up, Wv_up, ln_g, ln_b, Wo):
    import ml_dtypes

    bf = ml_dtypes.bfloat16
    f = lambda a: np.ascontiguousarray(np.asarray(a, dtype=np.float32))
    fb = lambda a: np.ascontiguousarray(np.asarray(a, dtype=np.float32)).astype(bf)
    mask = _make_mask().astype(bf)
    maps = []
    for bi in range(2):
        xTb = f(np.asarray(x)[bi].T)
        lpTb = f(np.asarray(latent_prev)[bi].T)
        wd_f = f(Wdown)
        for hg in range(4):
            sl = slice(hg * 512, (hg + 1) * 512)
            maps.append(
                {
                    "xT": xTb,
                    "xq": np.ascontiguousarray(xTb[:, sl]),
                    "lpT": lpTb,
                    "wq": f(np.asarray(Wq)[:, sl]),
                    "wd": wd_f,
                    "wk": f(np.asarray(Wk_up)[:, sl]),
                    "wv": f(np.asarray(Wv_up)[:, sl]),
                    "wk16": fb(np.asarray(Wk_up)[:, sl]),
                    "wv16": fb(np.asarray(Wv_up)[:, sl]),
                    "wo": fb(np.asarray(Wo)[sl, :]),
                    "g": f(ln_g),
                    "bb": f(ln_b),
                    "mask": mask,
                }
            )
    return maps


def run(trace=False, **inputs):
    from concourse.bass_utils import run_bass_kernel_spmd

    nc = _get_nc()
    maps = _in_maps(**inputs)
    res = run_bass_kernel_spmd(nc, maps, core_ids=list(range(8)), trace=trace)
    outs = [res.results[c]["o"] for c in range(8)]
    out = np.stack(
        [
            outs[0] + outs[1] + outs[2] + outs[3],
            outs[4] + outs[5] + outs[6] + outs[7],
        ],
        axis=0,
    ).astype(np.float32)
    return out, res


def kernel(**inputs):
    out, _ = run(trace=False, **inputs)
    return out


# revision 23
# speedup vs baseline: 1.0174x; 1.0026x over previous
"""MultiHeadLatentAttention on 8 Trainium2 NeuronCores.

Sharding: 2 batches x 4 head-groups (4 heads each) = 8 cores.
Each core computes, for its batch b and heads [4*hg, 4*hg+4):
  q = x[b] @ Wq[:, cols]                  (computed transposed: qT [512, T])
  latent_new = LN(x[b] @ Wdown)           (computed transposed, replicated on
                                           the 4 cores of the same batch)
  kT = (latent @ Wk[:, cols]).T           v = latent @ Wv[:, cols]
  scores.T, softmax (no max-subtraction; |scores| <= ~3), PV accumulation
  o_partial = attn_out @ Wo[rows, :]      -> [T, D] partial sum
Host sums the 4 partials per batch and stacks the 2 batches.

Dtype strategy: fp32r (full-rate PE; its single-xbus LDWEIGHTS hides
fully: ~229 ns/matmul) for the projection phases; bf16 only for the
attention phase where SBUF capacity forces 2-byte K/V/q residency (bf16
pays ~+30 ns/matmul to FWL xbus contention).

Schedule notes: the down-projection runs before the q projection so the
tt=3 LayerNorm tail overlaps the q matmuls instead of stalling phase B.
x streams as [128,1024] tiles (8 matmuls per DMA descriptor) so the
sync-queue issue rate can keep the PE fed; weight/latent prefetches ride
the gpsimd DMA queue in parallel. V is computed for all 4 heads per
matmul (512-wide, full rate). The softmax denominator sums 4 exp'd
tiles on DVE per ones-matmul.
"""

import numpy as np

N_HEADS = 16
T = 2048
D = 2048
LAT = 512
PAST = 2048
S = PAST + T  # 4096, below the 8192 cache cap
HD = D // N_HEADS  # 128
HPC = 4  # heads per core
LN_EPS = 1e-5
SCALE = 1.0 / float(np.sqrt(HD))
NJB = S // 128  # 32 key blocks
NTT = T // 512  # 4 query tiles
NDC = D // 128  # 16
NLC = LAT // 128  # 4

_CACHE = {}


def _r(ap):
    import concourse.mybir as mybir

    return ap.bitcast(mybir.dt.float32r)


def _build():
    import concourse.bacc as bacc
    import concourse.mybir as mybir
    import concourse.tile as tile
    from concourse import bass_isa

    f32 = mybir.dt.float32
    f32r = mybir.dt.float32r
    bf16 = mybir.dt.bfloat16
    AF = mybir.ActivationFunctionType
    OP = mybir.AluOpType

    nc = bacc.Bacc("TRN2", target_bir_lowering=False, debug=False, num_devices=8)

    xT = nc.dram_tensor("xT", [D, T], f32, kind="ExternalInput")
    xq = nc.dram_tensor("xq", [D, 512], f32, kind="ExternalInput")
    lpT = nc.dram_tensor("lpT", [LAT, PAST], f32, kind="ExternalInput")
    wq = nc.dram_tensor("wq", [D, LAT], f32, kind="ExternalInput")
    wd = nc.dram_tensor("wd", [D, LAT], f32, kind="ExternalInput")
    wk = nc.dram_tensor("wk", [LAT, LAT], f32, kind="ExternalInput")
    wv = nc.dram_tensor("wv", [LAT, LAT], f32, kind="ExternalInput")
    wk16 = nc.dram_tensor("wk16", [LAT, LAT], bf16, kind="ExternalInput")
    wv16 = nc.dram_tensor("wv16", [LAT, LAT], bf16, kind="ExternalInput")
    wo = nc.dram_tensor("wo", [LAT, D], bf16, kind="ExternalInput")
    g = nc.dram_tensor("g", [LAT], f32, kind="ExternalInput")
    b = nc.dram_tensor("bb", [LAT], f32, kind="ExternalInput")
    mask = nc.dram_tensor("mask", [128, 896], bf16, kind="ExternalInput")
    o = nc.dram_tensor("o", [T, D], f32, kind="ExternalOutput")

    with tile.TileContext(nc) as tc:
        with (
            tc.tile_pool(name="consts", bufs=1) as consts,
            tc.tile_pool(name="persist", bufs=1) as persist,
            tc.tile_pool(name="ccd", bufs=1, space="DRAM") as ccd,
        ):
            # mask[jj, c] = 1 iff c >= jj + 384, so cols [512, 640) are all
            # ones for every partition — doubles as the ones matrix for the
            # softmax-denominator matmul.
            mask_sb = consts.tile([128, 896], bf16)
            nc.sync.dma_start(out=mask_sb, in_=mask[:, :])
            ones_sb = mask_sb[:, 512:640]
            g_sb = consts.tile([128, NLC], f32)
            nc.sync.dma_start(out=g_sb, in_=g[:].rearrange("(lc p) -> p lc", p=128))
            b_sb = consts.tile([128, NLC], f32)
            nc.sync.dma_start(out=b_sb, in_=b[:].rearrange("(lc p) -> p lc", p=128))
            eps_sb = consts.tile([128, 1], f32)
            nc.vector.memset(eps_sb, LN_EPS)

            qT_sb = persist.tile([128, HPC, T], bf16)  # q transposed, per head
            latn_sb = persist.tile([128, NLC, T], bf16)  # new latent (gathered)
            ao_sb = persist.tile([128, HPC, T], bf16)  # attn out transposed
            wk_sb = persist.tile([128, NLC, LAT], f32r)
            wv_sb = persist.tile([128, NLC, LAT], f32r)
            wk16_sb = persist.tile([128, NLC, LAT], bf16)
            wv16_sb = persist.tile([128, NLC, LAT], bf16)
            # k/v weights ride the gpsimd DMA queue, in parallel with the
            # sync-queue weight/x streams of phase A
            nc.gpsimd.dma_start(
                wk_sb[:], wk[:, :].rearrange("(lc p) n -> p lc n", p=128).bitcast(f32r)
            )
            nc.gpsimd.dma_start(
                wv_sb[:], wv[:, :].rearrange("(lc p) n -> p lc n", p=128).bitcast(f32r)
            )
            nc.gpsimd.dma_start(
                wk16_sb[:], wk16[:, :].rearrange("(lc p) n -> p lc n", p=128)
            )
            nc.gpsimd.dma_start(
                wv16_sb[:], wv16[:, :].rearrange("(lc p) n -> p lc n", p=128)
            )
            cc_in = ccd.tile([LAT, 512], bf16)
            cc_out = ccd.tile([4 * LAT, 512], bf16)


            # ---- Phase A: latent down-projection + LayerNorm (first, so the
            # LN tail overlaps the q matmuls), then q projection.
            with (
                tc.tile_pool(name="wA", bufs=1) as wA,
                tc.tile_pool(name="xa", bufs=5) as xa,
                tc.tile_pool(name="stats", bufs=1) as stats,
                tc.tile_pool(name="psA", bufs=1, space="PSUM") as psA,
            ):
                wq_sb = wA.tile([128, NDC, LAT], f32r)
                wd_sb = wA.tile([128, NDC, LAT], f32r)
                for dc in range(4):
                    nc.sync.dma_start(
                        out=wd_sb[:, dc, :],
                        in_=wd[dc * 128 : (dc + 1) * 128, :].bitcast(f32r),
                    )

                latqb_sb = wA.tile([128, NLC, 512], bf16)

                def ln_ops_shard():
                    # LayerNorm over this core's latent quarter, staged as
                    # single-op thunks interleaved into the q-pass stream.
                    # Final scale+shift writes bf16 (the AllGather payload).
                    down = None
                    box = {}
                    ops = []

                    def alloc(nm, shape, tag):
                        box[nm] = stats.tile([128, shape], f32, tag=tag, name=nm)
                        return box[nm]

                    def dn(lc):
                        return latq_sb[:, lc, :]

                    ops.append(lambda: nc.vector.tensor_add(
                        alloc("ss2", 1024, "ss2")[:, 0:512], dn(0), dn(1)))
                    ops.append(lambda: nc.vector.tensor_add(
                        box["ss2"][:, 0:512], box["ss2"][:, 0:512], dn(2)))
                    ops.append(lambda: nc.vector.tensor_add(
                        box["ss2"][:, 0:512], box["ss2"][:, 0:512], dn(3)))
                    ops.append(lambda: nc.scalar.square(
                        alloc("sqa", 512, "sqa"), dn(0)))
                    ops.append(lambda: nc.scalar.square(
                        alloc("sqb", 512, "sqb"), dn(1)))
                    ops.append(lambda: nc.vector.tensor_add(
                        box["ss2"][:, 512:1024], box["sqa"], box["sqb"]))
                    ops.append(lambda: nc.scalar.square(
                        alloc("sqa2", 512, "sqa"), dn(2)))
                    ops.append(lambda: nc.vector.tensor_add(
                        box["ss2"][:, 512:1024], box["ss2"][:, 512:1024],
                        box["sqa2"]))
                    ops.append(lambda: nc.scalar.square(
                        alloc("sqb2", 512, "sqb"), dn(3)))
                    ops.append(lambda: nc.vector.tensor_add(
                        box["ss2"][:, 512:1024], box["ss2"][:, 512:1024],
                        box["sqb2"]))
                    ops.append(lambda: nc.gpsimd.partition_all_reduce(
                        alloc("pr2", 1024, "pr2"), box["ss2"], channels=128,
                        reduce_op=bass_isa.ReduceOp.add))
                    ops.append(lambda: nc.vector.tensor_scalar_mul(
                        alloc("mu", 512, "mu"), box["pr2"][:, 0:512], 1.0 / LAT))
                    ops.append(lambda: nc.vector.tensor_mul(
                        alloc("vtmp", 512, "vtmp"), box["mu"], box["mu"]))
                    ops.append(lambda: nc.vector.scalar_tensor_tensor(
                        out=alloc("sd", 512, "sd"), in0=box["pr2"][:, 512:1024],
                        scalar=1.0 / LAT, in1=box["vtmp"], op0=OP.mult,
                        op1=OP.subtract))
                    ops.append(lambda: nc.scalar.activation(
                        box["sd"], box["sd"], AF.Sqrt, bias=eps_sb))
                    ops.append(lambda: nc.vector.reciprocal_approx_fast(
                        alloc("rstd", 512, "rstd"), box["sd"]))
                    for lc in range(NLC):
                        ops.append(lambda lc=lc: nc.vector.tensor_sub(
                            alloc("t1", 512, "sqa"), dn(lc), box["mu"]))
                        ops.append(lambda: nc.vector.tensor_mul(
                            alloc("t2", 512, "sqb"), box["t1"], box["rstd"]))
                        ops.append(lambda lc=lc: nc.vector.tensor_scalar(
                            latqb_sb[:, lc, :], box["t2"], g_sb[:, lc : lc + 1],
                            b_sb[:, lc : lc + 1], OP.mult, OP.add))
                    return ops

                pend_ln = []

                def drain_ln(k):
                    for _ in range(min(k, len(pend_ln))):
                        pend_ln.pop(0)()

                # down-projection for this core's T/4 token quarter
                # (64 matmuls); the full new latent arrives via AllGather.
                latq_sb = wA.tile([128, NLC, 512], f32)
                d_ps = [
                    psA.tile([128, 512], f32, tag=f"t{i}", name=f"d_ps{i}")
                    for i in range(NLC)
                ]
                for dc in range(NDC):
                    xtq = xa.tile([128, 512], f32r, tag="xq", bufs=3, name="xtq")
                    nc.sync.dma_start(
                        out=xtq, in_=xq[dc * 128 : (dc + 1) * 128, :].bitcast(f32r)
                    )
                    if dc < 12:
                        nc.sync.dma_start(
                            out=wd_sb[:, dc + 4, :],
                            in_=wd[(dc + 4) * 128 : (dc + 5) * 128, :].bitcast(
                                f32r
                            ),
                        )
                    for lc in range(NLC):
                        nc.tensor.matmul(
                            d_ps[lc],
                            lhsT=_r(wd_sb[:, dc, lc * 128 : (lc + 1) * 128]),
                            rhs=_r(xtq),
                            start=(dc == 0),
                            stop=(dc == NDC - 1),
                        )
                for lc in range(NLC):
                    if lc % 2 == 0:
                        nc.vector.tensor_copy(latq_sb[:, lc, :], d_ps[lc])
                    else:
                        nc.scalar.copy(out=latq_sb[:, lc, :], in_=d_ps[lc])
                for dc in range(4):
                    nc.sync.dma_start(
                        out=wq_sb[:, dc, :],
                        in_=wq[dc * 128 : (dc + 1) * 128, :].bitcast(f32r),
                    )
                pend_ln.extend(ln_ops_shard())

                # q projection (re-streams x; wq chunks 4..15 stream behind)
                for tp in range(2):
                    psl = slice(tp * 1024, (tp + 1) * 1024)
                    q_ps = [
                        psA.tile([128, 512], f32, tag=f"t{i}", name=f"q_ps{i}")
                        for i in range(8)
                    ]
                    for dc in range(NDC):
                        xt = xa.tile([128, 1024], f32r, tag="xt", name="xt")
                        nc.sync.dma_start(
                            out=xt,
                            in_=xT[dc * 128 : (dc + 1) * 128, psl].bitcast(f32r),
                        )
                        if tp == 0 and dc < 12:
                            nc.sync.dma_start(
                                out=wq_sb[:, dc + 4, :],
                                in_=wq[(dc + 4) * 128 : (dc + 5) * 128, :].bitcast(
                                    f32r
                                ),
                            )
                        drain_ln(4)
                        for half in range(2):
                            for qc in range(HPC):
                                nc.tensor.matmul(
                                    q_ps[half * 4 + qc],
                                    lhsT=_r(wq_sb[:, dc, qc * 128 : (qc + 1) * 128]),
                                    rhs=_r(xt[:, half * 512 : (half + 1) * 512]),
                                    start=(dc == 0),
                                    stop=(dc == NDC - 1),
                                )
                    for i in range(8):
                        half, qc = i // 4, i % 4
                        tt = tp * 2 + half
                        dst = qT_sb[:, qc, tt * 512 : (tt + 1) * 512]
                        if i % 2 == 0:
                            nc.scalar.copy(out=dst, in_=q_ps[i])
                        else:
                            nc.vector.tensor_copy(dst, q_ps[i])
                    if tp == 0:
                        drain_ln(999)
                        # ship this core's normalized latent quarter; gather
                        # the batch group's four quarters into latn_sb. The
                        # collective overlaps the rest of phase A and B-past.
                        nc.gpsimd.dma_start(
                            cc_in[:].rearrange("(lc p) t -> p lc t", p=128),
                            latqb_sb[:],
                        )
                        nc.gpsimd.collective_compute(
                            "AllGather",
                            mybir.AluOpType.bypass,
                            replica_groups=[[0, 1, 2, 3], [4, 5, 6, 7]],
                            ins=[cc_in.opt()],
                            outs=[cc_out.opt()],
                        )
                        for rk in range(4):
                            nc.gpsimd.dma_start(
                                latn_sb[:, :, rk * 512 : (rk + 1) * 512],
                                cc_out[rk * 512 : (rk + 1) * 512, :].rearrange(
                                    "(lc p) t -> p lc t", p=128
                                ),
                            )

            # ---- Phase B: k/v up-projection for all 4 heads
            with tc.tile_pool(name="kvbuf", bufs=1) as kvp:
                kT_sb = kvp.tile([128, HPC, S], bf16)
                v_sb = kvp.tile([128, NJB, LAT], bf16)
                with (
                    tc.tile_pool(name="bbuf", bufs=1) as bst,
                    tc.tile_pool(name="psB", bufs=1, space="PSUM") as psB,
                ):
                    lp_sb = bst.tile([128, NLC, PAST], f32r)

                    def latf(lc, g_):
                        if g_ < PAST // 512:
                            return lp_sb[:, lc, g_ * 512 : (g_ + 1) * 512]
                        gg = g_ - PAST // 512
                        return latn_sb[:, lc, gg * 512 : (gg + 1) * 512]

                    def wk_of(g_, lc, hsl):
                        if g_ < PAST // 512:
                            return _r(wk_sb[:, lc, hsl])
                        return wk16_sb[:, lc, hsl]

                    def wv_of(g_, lc):
                        if g_ < PAST // 512:
                            return _r(wv_sb[:, lc, :])
                        return wv16_sb[:, lc, :]

                    def lat_of(g_, lc, j4=None):
                        ap = latf(lc, g_)
                        if j4 is not None:
                            ap = ap[:, j4 * 128 : (j4 + 1) * 128]
                        if g_ < PAST // 512:
                            return _r(ap)
                        return ap

                    for g_ in range(S // 512):
                        ssl = slice(g_ * 512, (g_ + 1) * 512)
                        if g_ < PAST // 512:
                            # past-latent chunks stream on the sync queue
                            # (gpsimd is owned by the collective right now)
                            for lc in range(NLC):
                                nc.sync.dma_start(
                                    out=lp_sb[:, lc, ssl],
                                    in_=lpT[lc * 128 : (lc + 1) * 128, ssl].bitcast(
                                        f32r
                                    ),
                                )
                        for h in range(HPC):
                            hsl = slice(h * 128, (h + 1) * 128)
                            k_ps = psB.tile(
                                [128, 512], f32, tag="kps", bufs=3, name="k_ps"
                            )
                            for lc in range(NLC):
                                nc.tensor.matmul(
                                    k_ps,
                                    lhsT=wk_of(g_, lc, hsl),
                                    rhs=lat_of(g_, lc),
                                    start=(lc == 0),
                                    stop=(lc == NLC - 1),
                                )
                            if h % 2 == 0:
                                nc.vector.tensor_copy(kT_sb[:, h, ssl], k_ps)
                            else:
                                nc.scalar.copy(out=kT_sb[:, h, ssl], in_=k_ps)
                        for j4 in range(4):
                            v_ps = psB.tile(
                                [128, 512], f32, tag="vps", bufs=3, name="v_ps"
                            )
                            for lc in range(NLC):
                                nc.tensor.matmul(
                                    v_ps,
                                    lhsT=lat_of(g_, lc, j4),
                                    rhs=wv_of(g_, lc),
                                    start=(lc == 0),
                                    stop=(lc == NLC - 1),
                                )
                            if j4 % 2 == 0:
                                nc.scalar.copy(out=v_sb[:, g_ * 4 + j4, :], in_=v_ps)
                            else:
                                nc.vector.tensor_copy(v_sb[:, g_ * 4 + j4, :], v_ps)

                # ---- Phase C: attention per head / query tile
                with (
                    tc.tile_pool(name="pp", bufs=11) as pp,
                    tc.tile_pool(name="pdp", bufs=3) as pdp,
                    tc.tile_pool(name="ctmp", bufs=2) as ctmp,
                    tc.tile_pool(name="psC", bufs=1, space="PSUM") as psC,
                ):
                  wo_sb = kvp.tile([128, HPC, D], bf16)
                  nc.sync.dma_start(
                      out=wo_sb[:],
                      in_=wo[:, :].rearrange("(hc p) n -> p hc n", p=128),
                  )
                  for h in range(HPC):
                    hsl = slice(h * 128, (h + 1) * 128)
                    for tt in range(NTT):
                        tsl = slice(tt * 512, (tt + 1) * 512)
                        nvis = PAST // 128 + 4 * (tt + 1)  # 20/24/28/32
                        # attention accumulator and softmax denominator share
                        # one psum tile (same lifetime)
                        ad = psC.tile([128, 1024], f32, tag="ad", bufs=2, name="ad")
                        attn_ps = ad[:, 0:512]
                        den_ps = ad[:, 512:1024]
                        pend = []
                        den_started = [False]
                        for jb in range(nvis):
                            s_ps = psC.tile(
                                [128, 512], f32, tag="sps", bufs=4, name="s_ps"
                            )
                            nc.tensor.matmul(
                                s_ps,
                                lhsT=kT_sb[:, h, jb * 128 : (jb + 1) * 128],
                                rhs=qT_sb[:, h, tsl],
                                start=True,
                                stop=True,
                            )
                            p = pp.tile([128, 512], bf16, tag="p", name="p")
                            nc.scalar.activation(p, s_ps, AF.Exp, scale=SCALE)
                            jbn = jb - PAST // 128
                            if jbn >= 0 and jbn // 4 == tt:
                                rr = (jbn % 4) * 128
                                nc.vector.tensor_mul(
                                    p, p, mask_sb[:, 384 - rr : 896 - rr]
                                )
                            nc.tensor.matmul(
                                attn_ps,
                                lhsT=v_sb[:, jb, hsl],
                                rhs=p,
                                start=(jb == 0),
                                stop=(jb == nvis - 1),
                            )
                            # denominator: accumulate up to 8 exp'd tiles
                            # on DVE per ones-matmul
                            pend.append(p)
                            if len(pend) == 8 or jb == nvis - 1:
                                e = pdp.tile([128, 512], bf16, tag="pd", name="e")
                                nc.vector.tensor_add(e, pend[0], pend[1])
                                for pk in pend[2:]:
                                    nc.vector.tensor_add(e, e, pk)
                                nc.tensor.matmul(
                                    den_ps,
                                    lhsT=ones_sb,
                                    rhs=e,
                                    start=not den_started[0],
                                    stop=(jb == nvis - 1),
                                )
                                den_started[0] = True
                                pend.clear()
                        rec = ctmp.tile([128, 512], f32, tag="rec", name="rec")
                        nc.vector.reciprocal_approx_fast(rec, den_ps)
                        nc.vector.tensor_mul(ao_sb[:, h, tsl], attn_ps, rec)

                # ---- Phase D: output projection (bf16: ao and wo)
                with (
                    tc.tile_pool(name="ost", bufs=4) as ost,
                    tc.tile_pool(name="psD", bufs=4, space="PSUM") as psD,
                ):
                    for dt_ in range(D // 512):
                        for tc_ in range(T // 128):
                            o_ps = psD.tile([128, 512], f32, tag="ops", name="o_ps")
                            for hc in range(HPC):
                                nc.tensor.matmul(
                                    o_ps,
                                    lhsT=ao_sb[:, hc, tc_ * 128 : (tc_ + 1) * 128],
                                    rhs=wo_sb[:, hc, dt_ * 512 : (dt_ + 1) * 512],
                                    start=(hc == 0),
                                    stop=(hc == HPC - 1),
                                )
                            o_sb = ost.tile([128, 512], f32, tag="osb", name="o_sb")
                            if tc_ % 2 == 0:
                                nc.scalar.copy(out=o_sb, in_=o_ps)
                            else:
                                nc.vector.tensor_copy(o_sb, o_ps)
                            nc.sync.dma_start(
                                out=o[
                                    tc_ * 128 : (tc_ + 1) * 128,
                                    dt_ * 512 : (dt_ + 1) * 512,
                                ],
                                in_=o_sb,
                            )

    nc.compile()
    return nc


def _get_nc():
    if "nc" not in _CACHE:
        _CACHE["nc"] = _build()
    return _CACHE["nc"]


def _make_mask():
    # B[jj, c] = 1.0 iff c >= jj + 384; sliced at 384-r it gives the
    # causal staircase "visible iff i >= jj + r" for r in {0,128,256,384}.
    jj = np.arange(128)[:, None]
    cc = np.arange(896)[None, :]
    return (cc >= jj + 384)


def _in_maps(x, latent_prev, Wq, Wdown, Wk_up, Wv_up, ln_g, ln_b, Wo):
    import ml_dtypes

    bf = ml_dtypes.bfloat16
    f = lambda a: np.ascontiguousarray(np.asarray(a, dtype=np.float32))
    fb = lambda a: np.ascontiguousarray(np.asarray(a, dtype=np.float32)).astype(bf)
    mask = _make_mask().astype(bf)
    maps = []
    for bi in range(2):
        xTb = f(np.asarray(x)[bi].T)
        lpTb = f(np.asarray(latent_prev)[bi].T)
        wd_f = f(Wdown)
        for hg in range(4):
            sl = slice(hg * 512, (hg + 1) * 512)
            maps.append(
                {
                    "xT": xTb,
                    "xq": np.ascontiguousarray(xTb[:, sl]),
                    "lpT": lpTb,
                    "wq": f(np.asarray(Wq)[:, sl]),
                    "wd": wd_f,
                    "wk": f(np.asarray(Wk_up)[:, sl]),
                    "wv": f(np.asarray(Wv_up)[:, sl]),
                    "wk16": fb(np.asarray(Wk_up)[:, sl]),
                    "wv16": fb(np.asarray(Wv_up)[:, sl]),
                    "wo": fb(np.asarray(Wo)[sl, :]),
                    "g": f(ln_g),
                    "bb": f(ln_b),
                    "mask": mask,
                }
            )
    return maps


def run(trace=False, **inputs):
    from concourse.bass_utils import run_bass_kernel_spmd

    nc = _get_nc()
    maps = _in_maps(**inputs)
    res = run_bass_kernel_spmd(nc, maps, core_ids=list(range(8)), trace=trace)
    outs = [res.results[c]["o"] for c in range(8)]
    out = np.stack(
        [
            outs[0] + outs[1] + outs[2] + outs[3],
            outs[4] + outs[5] + outs[6] + outs[7],
        ],
        axis=0,
    ).astype(np.float32)
    return out, res


def kernel(**inputs):
    out, _ = run(trace=False, **inputs)
    return out


# revision 25
# speedup vs baseline: 1.0428x; 1.0249x over previous
"""MultiHeadLatentAttention on 8 Trainium2 NeuronCores.

Sharding: 2 batches x 4 head-groups (4 heads each) = 8 cores.
Each core computes, for its batch b and heads [4*hg, 4*hg+4):
  q = x[b] @ Wq[:, cols]                  (computed transposed: qT [512, T])
  latent_new = LN(x[b] @ Wdown)           (computed transposed, replicated on
                                           the 4 cores of the same batch)
  kT = (latent @ Wk[:, cols]).T           v = latent @ Wv[:, cols]
  scores.T, softmax (no max-subtraction; |scores| <= ~3), PV accumulation
  o_partial = attn_out @ Wo[rows, :]      -> [T, D] partial sum
Host sums the 4 partials per batch and stacks the 2 batches.

Dtype strategy: fp32r (full-rate PE; its single-xbus LDWEIGHTS hides
fully: ~229 ns/matmul) for the projection phases; bf16 only for the
attention phase where SBUF capacity forces 2-byte K/V/q residency (bf16
pays ~+30 ns/matmul to FWL xbus contention).

Schedule notes: the down-projection runs before the q projection so the
tt=3 LayerNorm tail overlaps the q matmuls instead of stalling phase B.
x streams as [128,1024] tiles (8 matmuls per DMA descriptor) so the
sync-queue issue rate can keep the PE fed; weight/latent prefetches ride
the gpsimd DMA queue in parallel. V is computed for all 4 heads per
matmul (512-wide, full rate). The softmax denominator sums 4 exp'd
tiles on DVE per ones-matmul.
"""

import numpy as np

N_HEADS = 16
T = 2048
D = 2048
LAT = 512
PAST = 2048
S = PAST + T  # 4096, below the 8192 cache cap
HD = D // N_HEADS  # 128
HPC = 4  # heads per core
LN_EPS = 1e-5
SCALE = 1.0 / float(np.sqrt(HD))
NJB = S // 128  # 32 key blocks
NTT = T // 512  # 4 query tiles
NDC = D // 128  # 16
NLC = LAT // 128  # 4

_CACHE = {}


def _r(ap):
    import concourse.mybir as mybir

    return ap.bitcast(mybir.dt.float32r)


def _build():
    import concourse.bacc as bacc
    import concourse.mybir as mybir
    import concourse.tile as tile
    from concourse import bass_isa

    f32 = mybir.dt.float32
    f32r = mybir.dt.float32r
    bf16 = mybir.dt.bfloat16
    AF = mybir.ActivationFunctionType
    OP = mybir.AluOpType

    nc = bacc.Bacc("TRN2", target_bir_lowering=False, debug=False, num_devices=8)

    xT = nc.dram_tensor("xT", [D, T], f32, kind="ExternalInput")
    xq = nc.dram_tensor("xq", [D, 512], f32, kind="ExternalInput")
    lpT = nc.dram_tensor("lpT", [LAT, PAST], f32, kind="ExternalInput")
    wq = nc.dram_tensor("wq", [D, LAT], f32, kind="ExternalInput")
    wd = nc.dram_tensor("wd", [D, LAT], f32, kind="ExternalInput")
    wk = nc.dram_tensor("wk", [LAT, LAT], f32, kind="ExternalInput")
    wv = nc.dram_tensor("wv", [LAT, LAT], f32, kind="ExternalInput")
    wk16 = nc.dram_tensor("wk16", [LAT, LAT], bf16, kind="ExternalInput")
    wv16 = nc.dram_tensor("wv16", [LAT, LAT], bf16, kind="ExternalInput")
    wo = nc.dram_tensor("wo", [LAT, D], bf16, kind="ExternalInput")
    g = nc.dram_tensor("g", [LAT], f32, kind="ExternalInput")
    b = nc.dram_tensor("bb", [LAT], f32, kind="ExternalInput")
    mask = nc.dram_tensor("mask", [128, 896], bf16, kind="ExternalInput")
    o = nc.dram_tensor("o", [T, D], f32, kind="ExternalOutput")

    with tile.TileContext(nc) as tc:
        with (
            tc.tile_pool(name="consts", bufs=1) as consts,
            tc.tile_pool(name="persist", bufs=1) as persist,
            tc.tile_pool(name="ccd", bufs=1, space="DRAM") as ccd,
        ):
            # mask[jj, c] = 1 iff c >= jj + 384, so cols [512, 640) are all
            # ones for every partition — doubles as the ones matrix for the
            # softmax-denominator matmul.
            mask_sb = consts.tile([128, 896], bf16)
            nc.sync.dma_start(out=mask_sb, in_=mask[:, :])
            ones_sb = mask_sb[:, 512:640]
            g_sb = consts.tile([128, NLC], f32)
            nc.sync.dma_start(out=g_sb, in_=g[:].rearrange("(lc p) -> p lc", p=128))
            b_sb = consts.tile([128, NLC], f32)
            nc.sync.dma_start(out=b_sb, in_=b[:].rearrange("(lc p) -> p lc", p=128))
            eps_sb = consts.tile([128, 1], f32)
            nc.vector.memset(eps_sb, LN_EPS)
            ones_f32 = consts.tile([128, 128], f32)
            nc.vector.memset(ones_f32, 1.0)

            qT_sb = persist.tile([128, HPC, T], bf16)  # q transposed, per head
            latn_sb = persist.tile([128, NLC, T], bf16)  # new latent (gathered)
            ao_sb = persist.tile([128, HPC, T], bf16)  # attn out transposed
            wk_sb = persist.tile([128, NLC, LAT], f32r)
            wv_sb = persist.tile([128, NLC, LAT], f32r)
            wk16_sb = persist.tile([128, NLC, LAT], bf16)
            wv16_sb = persist.tile([128, NLC, LAT], bf16)
            # k/v weights ride the gpsimd DMA queue, in parallel with the
            # sync-queue weight/x streams of phase A
            nc.gpsimd.dma_start(
                wk_sb[:], wk[:, :].rearrange("(lc p) n -> p lc n", p=128).bitcast(f32r)
            )
            nc.gpsimd.dma_start(
                wv_sb[:], wv[:, :].rearrange("(lc p) n -> p lc n", p=128).bitcast(f32r)
            )
            nc.gpsimd.dma_start(
                wk16_sb[:], wk16[:, :].rearrange("(lc p) n -> p lc n", p=128)
            )
            nc.gpsimd.dma_start(
                wv16_sb[:], wv16[:, :].rearrange("(lc p) n -> p lc n", p=128)
            )
            cc_in = ccd.tile([LAT, 512], bf16)
            cc_out = ccd.tile([4 * LAT, 512], bf16)


            # ---- Phase A: latent down-projection + LayerNorm (first, so the
            # LN tail overlaps the q matmuls), then q projection.
            with (
                tc.tile_pool(name="wA", bufs=1) as wA,
                tc.tile_pool(name="xa", bufs=5) as xa,
                tc.tile_pool(name="stats", bufs=1) as stats,
                tc.tile_pool(name="psA", bufs=1, space="PSUM") as psA,
            ):
                wq_sb = wA.tile([128, NDC, LAT], f32r)
                wd_sb = wA.tile([128, NDC, LAT], f32r)
                for dc in range(4):
                    nc.sync.dma_start(
                        out=wd_sb[:, dc, :],
                        in_=wd[dc * 128 : (dc + 1) * 128, :].bitcast(f32r),
                    )

                latqb_sb = wA.tile([128, NLC, 512], bf16)

                def ln_emit_stats():
                    # LN stats via ones-matmuls into PSUM (t4/t5) — keeps
                    # gpsimd out of phase A so the pool-close barrier cannot
                    # queue behind the collective on the gpsimd queue.
                    ssum = psA.tile([128, 512], f32, tag="t4", name="ssum")
                    ssq = psA.tile([128, 512], f32, tag="t5", name="ssq")
                    sqs = []
                    for lc in range(NLC):
                        sq = stats.tile([128, 512], f32r, tag=f"sq{lc}", name="sq")
                        nc.scalar.square(sq, latq_sb[:, lc, :])
                        sqs.append(sq)
                    for lc in range(NLC):
                        nc.tensor.matmul(
                            ssum,
                            lhsT=_r(ones_f32),
                            rhs=latq_sb[:, lc, :],
                            start=(lc == 0),
                            stop=(lc == NLC - 1),
                        )
                    for lc in range(NLC):
                        nc.tensor.matmul(
                            ssq,
                            lhsT=_r(ones_f32),
                            rhs=sqs[lc],
                            start=(lc == 0),
                            stop=(lc == NLC - 1),
                        )
                    mu = stats.tile([128, 512], f32, tag="mu", name="mu")
                    nc.vector.tensor_scalar_mul(mu, ssum, 1.0 / LAT)
                    vtmp = stats.tile([128, 512], f32, tag="vtmp", name="vtmp")
                    nc.vector.tensor_mul(vtmp, mu, mu)
                    sd = stats.tile([128, 512], f32, tag="sd", name="sd")
                    nc.vector.scalar_tensor_tensor(
                        out=sd,
                        in0=ssq,
                        scalar=1.0 / LAT,
                        in1=vtmp,
                        op0=OP.mult,
                        op1=OP.subtract,
                    )
                    # normalize tail, staged into the q-pass stream
                    box = {"mu": mu, "sd": sd}
                    ops = []
                    ops.append(lambda: nc.scalar.activation(
                        box["sd"], box["sd"], AF.Sqrt, bias=eps_sb))

                    def mkrstd():
                        box["rstd"] = stats.tile(
                            [128, 512], f32, tag="rstd", name="rstd")
                        nc.vector.reciprocal_approx_fast(box["rstd"], box["sd"])

                    ops.append(mkrstd)
                    for lc in range(NLC):
                        def mk1(lc=lc):
                            box["t1"] = stats.tile(
                                [128, 512], f32, tag="sq0", name="t1")
                            nc.vector.tensor_sub(
                                box["t1"], latq_sb[:, lc, :], box["mu"])

                        def mk2():
                            box["t2"] = stats.tile(
                                [128, 512], f32, tag="sq1", name="t2")
                            nc.vector.tensor_mul(
                                box["t2"], box["t1"], box["rstd"])

                        ops.append(mk1)
                        ops.append(mk2)
                        ops.append(lambda lc=lc: nc.vector.tensor_scalar(
                            latqb_sb[:, lc, :], box["t2"], g_sb[:, lc : lc + 1],
                            b_sb[:, lc : lc + 1], OP.mult, OP.add))
                    return ops

                pend_ln = []

                def drain_ln(k):
                    for _ in range(min(k, len(pend_ln))):
                        pend_ln.pop(0)()

                # down-projection for this core's T/4 token quarter
                # (64 matmuls); the full new latent arrives via AllGather.
                latq_sb = wA.tile([128, NLC, 512], f32r)
                d_ps = [
                    psA.tile([128, 512], f32, tag=f"t{i}", name=f"d_ps{i}")
                    for i in range(NLC)
                ]
                for dc in range(NDC):
                    xtq = xa.tile([128, 512], f32r, tag="xq", bufs=3, name="xtq")
                    nc.sync.dma_start(
                        out=xtq, in_=xq[dc * 128 : (dc + 1) * 128, :].bitcast(f32r)
                    )
                    if dc < 12:
                        nc.sync.dma_start(
                            out=wd_sb[:, dc + 4, :],
                            in_=wd[(dc + 4) * 128 : (dc + 5) * 128, :].bitcast(
                                f32r
                            ),
                        )
                    for lc in range(NLC):
                        nc.tensor.matmul(
                            d_ps[lc],
                            lhsT=_r(wd_sb[:, dc, lc * 128 : (lc + 1) * 128]),
                            rhs=_r(xtq),
                            start=(dc == 0),
                            stop=(dc == NDC - 1),
                        )
                for lc in range(NLC):
                    if lc % 2 == 0:
                        nc.vector.tensor_copy(latq_sb[:, lc, :], d_ps[lc])
                    else:
                        nc.scalar.copy(out=latq_sb[:, lc, :], in_=d_ps[lc])
                for dc in range(4):
                    nc.sync.dma_start(
                        out=wq_sb[:, dc, :],
                        in_=wq[dc * 128 : (dc + 1) * 128, :].bitcast(f32r),
                    )
                pend_ln.extend(ln_emit_stats())

                # q projection (re-streams x; wq chunks 4..15 stream behind)
                for tp in range(2):
                    psl = slice(tp * 1024, (tp + 1) * 1024)
                    q_ps = [
                        psA.tile([128, 512], f32, tag=f"t{i}", name=f"q_ps{i}")
                        for i in range(8)
                    ]
                    for dc in range(NDC):
                        xt = xa.tile([128, 1024], f32r, tag="xt", name="xt")
                        nc.sync.dma_start(
                            out=xt,
                            in_=xT[dc * 128 : (dc + 1) * 128, psl].bitcast(f32r),
                        )
                        if tp == 0 and dc < 12:
                            nc.sync.dma_start(
                                out=wq_sb[:, dc + 4, :],
                                in_=wq[(dc + 4) * 128 : (dc + 5) * 128, :].bitcast(
                                    f32r
                                ),
                            )
                        drain_ln(4)
                        for half in range(2):
                            for qc in range(HPC):
                                nc.tensor.matmul(
                                    q_ps[half * 4 + qc],
                                    lhsT=_r(wq_sb[:, dc, qc * 128 : (qc + 1) * 128]),
                                    rhs=_r(xt[:, half * 512 : (half + 1) * 512]),
                                    start=(dc == 0),
                                    stop=(dc == NDC - 1),
                                )
                    for i in range(8):
                        half, qc = i // 4, i % 4
                        tt = tp * 2 + half
                        dst = qT_sb[:, qc, tt * 512 : (tt + 1) * 512]
                        if i % 2 == 0:
                            nc.scalar.copy(out=dst, in_=q_ps[i])
                        else:
                            nc.vector.tensor_copy(dst, q_ps[i])
                    if tp == 0:
                        drain_ln(999)
                        # ship this core's normalized latent quarter; gather
                        # the batch group's four quarters into latn_sb. The
                        # collective overlaps the rest of phase A and B-past.
                        nc.sync.dma_start(
                            out=cc_in[:].rearrange("(lc p) t -> p lc t", p=128),
                            in_=latqb_sb[:],
                        )
                        nc.gpsimd.collective_compute(
                            "AllGather",
                            mybir.AluOpType.bypass,
                            replica_groups=[[0, 1, 2, 3], [4, 5, 6, 7]],
                            ins=[cc_in.opt()],
                            outs=[cc_out.opt()],
                        )
                        for rk in range(4):
                            nc.gpsimd.dma_start(
                                latn_sb[:, :, rk * 512 : (rk + 1) * 512],
                                cc_out[rk * 512 : (rk + 1) * 512, :].rearrange(
                                    "(lc p) t -> p lc t", p=128
                                ),
                            )

            # ---- Phase B: k/v up-projection for all 4 heads
            with tc.tile_pool(name="kvbuf", bufs=1) as kvp:
                kT_sb = kvp.tile([128, HPC, S], bf16)
                v_sb = kvp.tile([128, NJB, LAT], bf16)
                with (
                    tc.tile_pool(name="bbuf", bufs=1) as bst,
                    tc.tile_pool(name="psB", bufs=1, space="PSUM") as psB,
                ):
                    lp_sb = bst.tile([128, NLC, PAST], f32r)

                    def latf(lc, g_):
                        if g_ < PAST // 512:
                            return lp_sb[:, lc, g_ * 512 : (g_ + 1) * 512]
                        gg = g_ - PAST // 512
                        return latn_sb[:, lc, gg * 512 : (gg + 1) * 512]

                    def wk_of(g_, lc, hsl):
                        if g_ < PAST // 512:
                            return _r(wk_sb[:, lc, hsl])
                        return wk16_sb[:, lc, hsl]

                    def wv_of(g_, lc):
                        if g_ < PAST // 512:
                            return _r(wv_sb[:, lc, :])
                        return wv16_sb[:, lc, :]

                    def lat_of(g_, lc, j4=None):
                        ap = latf(lc, g_)
                        if j4 is not None:
                            ap = ap[:, j4 * 128 : (j4 + 1) * 128]
                        if g_ < PAST // 512:
                            return _r(ap)
                        return ap

                    for g_ in range(S // 512):
                        ssl = slice(g_ * 512, (g_ + 1) * 512)
                        if g_ < PAST // 512:
                            # past-latent chunks stream on the sync queue
                            # (gpsimd is owned by the collective right now)
                            for lc in range(NLC):
                                nc.sync.dma_start(
                                    out=lp_sb[:, lc, ssl],
                                    in_=lpT[lc * 128 : (lc + 1) * 128, ssl].bitcast(
                                        f32r
                                    ),
                                )
                        for h in range(HPC):
                            hsl = slice(h * 128, (h + 1) * 128)
                            k_ps = psB.tile(
                                [128, 512], f32, tag="kps", bufs=3, name="k_ps"
                            )
                            for lc in range(NLC):
                                nc.tensor.matmul(
                                    k_ps,
                                    lhsT=wk_of(g_, lc, hsl),
                                    rhs=lat_of(g_, lc),
                                    start=(lc == 0),
                                    stop=(lc == NLC - 1),
                                )
                            if h % 2 == 0:
                                nc.vector.tensor_copy(kT_sb[:, h, ssl], k_ps)
                            else:
                                nc.scalar.copy(out=kT_sb[:, h, ssl], in_=k_ps)
                        for j4 in range(4):
                            v_ps = psB.tile(
                                [128, 512], f32, tag="vps", bufs=3, name="v_ps"
                            )
                            for lc in range(NLC):
                                nc.tensor.matmul(
                                    v_ps,
                                    lhsT=lat_of(g_, lc, j4),
                                    rhs=wv_of(g_, lc),
                                    start=(lc == 0),
                                    stop=(lc == NLC - 1),
                                )
                            if j4 % 2 == 0:
                                nc.scalar.copy(out=v_sb[:, g_ * 4 + j4, :], in_=v_ps)
                            else:
                                nc.vector.tensor_copy(v_sb[:, g_ * 4 + j4, :], v_ps)

                # ---- Phase C: attention per head / query tile
                with (
                    tc.tile_pool(name="pp", bufs=11) as pp,
                    tc.tile_pool(name="pdp", bufs=3) as pdp,
                    tc.tile_pool(name="ctmp", bufs=2) as ctmp,
                    tc.tile_pool(name="psC", bufs=1, space="PSUM") as psC,
                ):
                  wo_sb = kvp.tile([128, HPC, D], bf16)
                  nc.sync.dma_start(
                      out=wo_sb[:],
                      in_=wo[:, :].rearrange("(hc p) n -> p hc n", p=128),
                  )
                  for h in range(HPC):
                    hsl = slice(h * 128, (h + 1) * 128)
                    for tt in range(NTT):
                        tsl = slice(tt * 512, (tt + 1) * 512)
                        nvis = PAST // 128 + 4 * (tt + 1)  # 20/24/28/32
                        # attention accumulator and softmax denominator share
                        # one psum tile (same lifetime)
                        ad = psC.tile([128, 1024], f32, tag="ad", bufs=2, name="ad")
                        attn_ps = ad[:, 0:512]
                        den_ps = ad[:, 512:1024]
                        pend = []
                        den_started = [False]
                        for jb in range(nvis):
                            s_ps = psC.tile(
                                [128, 512], f32, tag="sps", bufs=4, name="s_ps"
                            )
                            nc.tensor.matmul(
                                s_ps,
                                lhsT=kT_sb[:, h, jb * 128 : (jb + 1) * 128],
                                rhs=qT_sb[:, h, tsl],
                                start=True,
                                stop=True,
                            )
                            p = pp.tile([128, 512], bf16, tag="p", name="p")
                            nc.scalar.activation(p, s_ps, AF.Exp, scale=SCALE)
                            jbn = jb - PAST // 128
                            if jbn >= 0 and jbn // 4 == tt:
                                rr = (jbn % 4) * 128
                                nc.vector.tensor_mul(
                                    p, p, mask_sb[:, 384 - rr : 896 - rr]
                                )
                            nc.tensor.matmul(
                                attn_ps,
                                lhsT=v_sb[:, jb, hsl],
                                rhs=p,
                                start=(jb == 0),
                                stop=(jb == nvis - 1),
                            )
                            # denominator: accumulate up to 8 exp'd tiles
                            # on DVE per ones-matmul
                            pend.append(p)
                            if len(pend) == 8 or jb == nvis - 1:
                                e = pdp.tile([128, 512], bf16, tag="pd", name="e")
                                nc.vector.tensor_add(e, pend[0], pend[1])
                                for pk in pend[2:]:
                                    nc.vector.tensor_add(e, e, pk)
                                nc.tensor.matmul(
                                    den_ps,
                                    lhsT=ones_sb,
                                    rhs=e,
                                    start=not den_started[0],
                                    stop=(jb == nvis - 1),
                                )
                                den_started[0] = True
                                pend.clear()
                        rec = ctmp.tile([128, 512], f32, tag="rec", name="rec")
                        nc.vector.reciprocal_approx_fast(rec, den_ps)
                        nc.vector.tensor_mul(ao_sb[:, h, tsl], attn_ps, rec)

                # ---- Phase D: output projection (bf16: ao and wo)
                with (
                    tc.tile_pool(name="ost", bufs=4) as ost,
                    tc.tile_pool(name="psD", bufs=4, space="PSUM") as psD,
                ):
                    for dt_ in range(D // 512):
                        for tc_ in range(T // 128):
                            o_ps = psD.tile([128, 512], f32, tag="ops", name="o_ps")
                            for hc in range(HPC):
                                nc.tensor.matmul(
                                    o_ps,
                                    lhsT=ao_sb[:, hc, tc_ * 128 : (tc_ + 1) * 128],
                                    rhs=wo_sb[:, hc, dt_ * 512 : (dt_ + 1) * 512],
                                    start=(hc == 0),
                                    stop=(hc == HPC - 1),
                                )
                            o_sb = ost.tile([128, 512], f32, tag="osb", name="o_sb")
                            if tc_ % 2 == 0:
                                nc.scalar.copy(out=o_sb, in_=o_ps)
                            else:
                                nc.vector.tensor_copy(o_sb, o_ps)
                            nc.sync.dma_start(
                                out=o[
                                    tc_ * 128 : (tc_ + 1) * 128,
                                    dt_ * 512 : (dt_ + 1) * 512,
                                ],
                                in_=o_sb,
                            )

    nc.compile()
    return nc


def _get_nc():
    if "nc" not in _CACHE:
        _CACHE["nc"] = _build()
    return _CACHE["nc"]


def _make_mask():
    # B[jj, c] = 1.0 iff c >= jj + 384; sliced at 384-r it gives the
    # causal staircase "visible iff i >= jj + r" for r in {0,128,256,384}.
    jj = np.arange(128)[:, None]
    cc = np.arange(896)[None, :]
    return (cc >= jj + 384)


def _in_maps(x, latent_prev, Wq, Wdown, Wk_up, Wv_up, ln_g, ln_b, Wo):
    import ml_dtypes

    bf = ml_dtypes.bfloat16
    f = lambda a: np.ascontiguousarray(np.asarray(a, dtype=np.float32))
    fb = lambda a: np.ascontiguousarray(np.asarray(a, dtype=np.float32)).astype(bf)
    mask = _make_mask().astype(bf)
    maps = []
    for bi in range(2):
        xTb = f(np.asarray(x)[bi].T)
        lpTb = f(np.asarray(latent_prev)[bi].T)
        wd_f = f(Wdown)
        for hg in range(4):
            sl = slice(hg * 512, (hg + 1) * 512)
            maps.append(
                {
                    "xT": xTb,
                    "xq": np.ascontiguousarray(xTb[:, sl]),
                    "lpT": lpTb,
                    "wq": f(np.asarray(Wq)[:, sl]),
                    "wd": wd_f,
                    "wk": f(np.asarray(Wk_up)[:, sl]),
                    "wv": f(np.asarray(Wv_up)[:, sl]),
                    "wk16": fb(np.asarray(Wk_up)[:, sl]),
                    "wv16": fb(np.asarray(Wv_up)[:, sl]),
                    "wo": fb(np.asarray(Wo)[sl, :]),
                    "g": f(ln_g),
                    "bb": f(ln_b),
                    "mask": mask,
                }
            )
    return maps


def run(trace=False, **inputs):
    from concourse.bass_utils import run_bass_kernel_spmd

    nc = _get_nc()
    maps = _in_maps(**inputs)
    res = run_bass_kernel_spmd(nc, maps, core_ids=list(range(8)), trace=trace)
    outs = [res.results[c]["o"] for c in range(8)]
    out = np.stack(
        [
            outs[0] + outs[1] + outs[2] + outs[3],
            outs[4] + outs[5] + outs[6] + outs[7],
        ],
        axis=0,
    ).astype(np.float32)
    return out, res


def kernel(**inputs):
    out, _ = run(trace=False, **inputs)
    return out
